# revision 39
# baseline (speedup 1.0000x reference)
"""Trainium2 Bass kernel for Gumbel 2:4-masked Linear (tensor-parallel over out_features).

Math (matches the reference in forward value):
  idx  = argmax over 6 logits per 4-weight block (logits = choice_weights +
         gumbel_noise; constant choice -> shift does not change the argmax,
         so it is skipped in 'const' mode)
  mask = MASKING_PATTERNS[idx]
  out  = x @ (weight * mask).T + bias

Distribution: 8 NeuronCores, sharded by output rows (512 rows/core). Mask
generation and the masked GEMM are fully local; outputs concatenated on host.

Numeric scheme (fp8 DoubleRow, ~4x bf16 matmul throughput in the cost model):
  x = (x8 + xr)/SX   w*mask = (w8 + wr)*mask/SW   (all planes float8e4)
  out*S = x8@(w8*M) + xr@(w8*M) + x8@(wr*M),  S = SX*SW = 4096
  Host rescales by 1/S after upcasting the bf16 device output. Measured
  end-to-end rel err ~1.1e-3 (full residual coverage), below bf16's 2.3e-3.
  COV_X/COV_W drop the residual products on trailing k-chunks to trade a
  little accuracy for PE/DMA time.

Layout (per core; k reordered globally as k'' = (bt, bi, p), b = bt*128+bi,
k_orig = 4*b + p — a pure permutation of the contraction dim, mirrored in the
host packing of x, so no device transposes at all):
  - g   [1024=(bt,bi), 6*512]  f32   planes ordered [g4,g3,g2,g1,g0,g5]
  - w8/wr [(bt,bi), 4*512] fp8       [p, o] per row, premultiplied by SW
  - xt  [128=bi, strip*(32+4*COV_X)*128] fp8  per strip: x8 k-tiles
        (bt-major, p-minor) then xr k-tiles
  - out [4096, 512] bf16             value = S*(x@wm^T + bias)

Mask gen per b-tile chunk (DVE maxes + is_equal, Pool does the two plane
multiplies; ACT free): S_p sets over original pattern index: col0={3,4,5}
col1={1,2,5} col2={0,2,4} col3={0,1,3}; with the plane order above the four
group maxes build from fused paired ops, then mask_p = (M_p == max6) exactly
(fp32 ties measure-zero), multiply w8/wr planes -> resident masked planes.

Schedule: 8 chunk groups; strip s admitted at chunk s (catch-up bursts for
chunks 0..s), strips 0..7 psum-resident (8 banks), strips 8..31 stream full
48-inst DoubleRow bursts over the resident masked planes; DVE drains psum +
bias -> bf16, out DMA from the DVE queue so the sync queue stays in
need-order for g/w/x.
"""

import numpy as np

N_CORES = 8
T = 4096
K = 4096
O_FULL = 4096
O = O_FULL // N_CORES           # 512 out rows per core
N_TT = T // 128                 # 32 token strips
N_BT = 8                        # k chunks (b-tiles); 512 k each
COV_X = 5                       # chunks with the xr@w8 residual product
COV_W = 8                       # chunks with the x8@wr residual product
SX = 16.0
SW = 256.0
S = SX * SW                     # psum scale, power of two
N_ADMIT = 8                     # psum-resident strips admitted during phase A

_prog_cache = {}


def _build_program(mode, cov_x=COV_X, cov_w=COV_W):
    """mode: 'const' (constant choice_weights folded away) or 'full'."""
    import concourse.bacc as bacc
    import concourse.bass as bass
    import concourse.mybir as mybir
    import concourse.tile as tile

    f32 = mybir.dt.float32
    bf16 = mybir.dt.bfloat16
    fp8 = mybir.dt.float8e4
    Alu = mybir.AluOpType
    DR = mybir.MatmulPerfMode.DoubleRow

    NKX = 32 + 4 * cov_x        # x k-tiles per strip (x8 + covered xr)

    nc = bacc.Bacc(trn_type="TRN2")
    xt_d = nc.declare_dram_parameter("xt", [128, N_TT * NKX * 128], fp8, isOutput=False)
    w8_d = nc.declare_dram_parameter("w8", [N_BT * 128, 4 * O], fp8, isOutput=False)
    wr_d = nc.declare_dram_parameter("wr", [cov_w * 128, 4 * O], fp8, isOutput=False)
    g_d = nc.declare_dram_parameter("g", [N_BT * 128, 6 * O], f32, isOutput=False)
    if mode == "full":
        cw_d = nc.declare_dram_parameter("cw", [N_BT * 128, 6 * O], f32, isOutput=False)
    b_d = nc.declare_dram_parameter("b", [1, O], f32, isOutput=False)
    out_d = nc.declare_dram_parameter("out", [T, O], bf16, isOutput=True)
    xt_v = xt_d.rearrange("p (s a t) -> p s a t", s=N_TT, a=NKX)

    with tile.TileContext(nc) as tc:
        with (
            tc.tile_pool(name="singles", bufs=1) as singles,
            tc.tile_pool(name="wm8", bufs=N_BT) as wm8_pool,
            tc.tile_pool(name="wmr", bufs=max(cov_w, 1)) as wmr_pool,
            tc.tile_pool(name="gum", bufs=3) as gum_pool,
            tc.tile_pool(name="wraw", bufs=4) as wraw_pool,
            tc.tile_pool(name="msk", bufs=3) as msk_pool,
            tc.tile_pool(name="mtmp", bufs=3) as mtmp,
            tc.tile_pool(name="xt", bufs=9) as xt_pool,
            tc.tile_pool(name="outs", bufs=4) as out_pool,
            tc.tile_pool(name="ps", bufs=8, space="PSUM") as ps_pool,
        ):
            bias_s = singles.tile([128, O], f32)
            nc.gpsimd.dma_start(
                out=bias_s,
                in_=bass.AP(tensor=b_d, offset=0, ap=[[0, 128], [1, O]]),
            )

            wm8 = [wm8_pool.tile([128, 4, O], fp8, name=f"wm8_{i}", tag=f"wm8_{i}",
                                 bufs=1) for i in range(N_BT)]
            wmr = [wmr_pool.tile([128, 4, O], fp8, name=f"wmr_{i}", tag=f"wmr_{i}",
                                 bufs=1) for i in range(cov_w)]

            g_tiles = {}
            w_tiles = {}
            x_tiles = {}

            def g_dma(bt, osl=None):
                rows = slice(bt * 128, (bt + 1) * 128)
                if bt not in g_tiles:
                    g_tiles[bt] = gum_pool.tile([128, 6, O], f32, tag="gum",
                                                name=f"g{bt}")
                g_t = g_tiles[bt]
                src = g_d[rows, :].rearrange("p (a b) -> p a b", a=6)
                if osl is None:
                    # planes 0-3 first: the leading DVE max op only reads
                    # those, so it can start ~1.5us before planes 4-5 land
                    nc.sync.dma_start(out=g_t[:, 0:4, :], in_=src[:, 0:4, :])
                    nc.sync.dma_start(out=g_t[:, 4:6, :], in_=src[:, 4:6, :])
                else:
                    nc.sync.dma_start(out=g_t[:, :, osl], in_=src[:, :, osl])
                if mode == "full" and (bt, "cw") not in g_tiles:
                    cw_t = gum_pool.tile([128, 6, O], f32, tag="cw", name=f"c{bt}")
                    nc.sync.dma_start(
                        out=cw_t, in_=cw_d[rows, :].rearrange("p (a b) -> p a b", a=6))
                    g_tiles[(bt, "cw")] = cw_t

            def w_dma(bt):
                rows = slice(bt * 128, (bt + 1) * 128)
                w8_t = wraw_pool.tile([128, 4, O], fp8, tag="w8raw", name=f"w8r{bt}")
                nc.sync.dma_start(
                    out=w8_t, in_=w8_d[rows, :].rearrange("p (a b) -> p a b", a=4))
                w_tiles[(bt, 8)] = w8_t
                if bt < cov_w:
                    wr_t = wraw_pool.tile([128, 4, O], fp8, tag="wrraw", name=f"wrr{bt}")
                    nc.sync.dma_start(
                        out=wr_t, in_=wr_d[rows, :].rearrange("p (a b) -> p a b", a=4))
                    w_tiles[(bt, "r")] = wr_t

            def x_dma(s):
                xs = xt_pool.tile([128, NKX, 128], fp8, tag="x", name=f"x{s}")
                nc.sync.dma_start(out=xs, in_=xt_v[:, s, :, :])
                x_tiles[s] = xs

            def mask_unit(bt, osl=None):
                """Masked fp8 weight planes for chunk bt (o-slice osl).

                Planes: 0=g4 1=g3 2=g2 3=g1 4=g0 5=g5. Group maxes via fused
                paired ops: u2=[M0|M1], u4=[M2|M3]; mask_p = (M_p == max6)
                on DVE (is_equal); plane multiplies on Pool (fp8 capable)."""
                osl = osl if osl is not None else slice(0, O)
                W = osl.stop - osl.start
                g_t = g_tiles[bt][:, :, osl]
                if mode == "full":
                    nc.vector.tensor_add(g_t, g_t, g_tiles[(bt, "cw")][:, :, osl])
                u2 = mtmp.tile([128, 2, W], f32, tag=f"u2_{W}", bufs=3 if W == O else 2)
                u4 = mtmp.tile([128, 2, W], f32, tag=f"u4_{W}", bufs=3 if W == O else 2)
                mx = mtmp.tile([128, W], f32, tag=f"mx_{W}", bufs=3 if W == O else 2)
                V = nc.vector
                V.tensor_tensor(u2, g_t[:, 1:4:2, :], g_t[:, 0:3:2, :], op=Alu.max)
                g5b = g_t[:, 5:6, :].broadcast_to([128, 2, W])
                V.tensor_tensor(u2, u2, g5b, op=Alu.max)            # [M0|M1]
                g0b = g_t[:, 4:5, :].broadcast_to([128, 2, W])
                V.tensor_tensor(u4, g0b, g_t[:, 2:4, :], op=Alu.max)
                V.tensor_tensor(u4, u4, g_t[:, 0:2, :], op=Alu.max)  # [M2|M3]
                V.tensor_tensor(mx, u2[:, 0, :], g_t[:, 2, :], op=Alu.max)
                V.tensor_tensor(mx, mx, u4[:, 1, :], op=Alu.max)
                mask = msk_pool.tile([128, 4, W], bf16, tag=f"mask_{W}", bufs=3 if W == O else 2)
                mxb = mx.rearrange("p (a b) -> p a b", a=1).broadcast_to([128, 2, W])
                V.tensor_tensor(mask[:, 0:2, :], u2, mxb, op=Alu.is_equal)
                V.tensor_tensor(mask[:, 2:4, :], u4, mxb, op=Alu.is_equal)
                nc.gpsimd.tensor_mul(wm8[bt][:, :, osl], w_tiles[(bt, 8)][:, :, osl],
                                     mask)
                if bt < cov_w:
                    nc.gpsimd.tensor_mul(wmr[bt][:, :, osl],
                                         w_tiles[(bt, "r")][:, :, osl], mask)

            # per-strip DoubleRow instruction budget (for start/stop flags)
            insts_per_strip = sum(
                2 + 2 * (bt < cov_x) + 2 * (bt < cov_w) for bt in range(N_BT))
            mm_count = {}
            acc_tiles = {}

            def burst(s, bt):
                """All DoubleRow insts of strip s against chunk bt's planes."""
                xs = x_tiles[s]
                if s not in acc_tiles:
                    acc_tiles[s] = ps_pool.tile([128, O], f32, tag="acc",
                                                name=f"acc{s}")
                    mm_count[s] = 0
                acc = acc_tiles[s]
                pairs = []
                for h in (0, 1):
                    pairs.append((bt * 4 + 2 * h, wm8[bt]))          # x8 @ w8*M
                if bt < cov_x:
                    for h in (0, 1):
                        pairs.append((32 + bt * 4 + 2 * h, wm8[bt]))  # xr @ w8*M
                if bt < cov_w:
                    for h in (0, 1):
                        pairs.append((bt * 4 + 2 * h, wmr[bt]))      # x8 @ wr*M
                for xoff, rhs_t in pairs:
                    h2 = (xoff % 4) // 2 * 2
                    nc.tensor.matmul(
                        acc,
                        xs[:, xoff:xoff + 2, :],
                        rhs_t[:, h2:h2 + 2, :],
                        start=(mm_count[s] == 0),
                        stop=(mm_count[s] == insts_per_strip - 1),
                        perf_mode=DR,
                    )
                    mm_count[s] += 1

            def drain(s):
                acc = acc_tiles.pop(s)
                o_t = out_pool.tile([128, O], bf16, tag="o", name=f"o{s}")
                nc.vector.tensor_add(o_t, acc, bias_s)
                nc.scalar.dma_start(out=out_d[s * 128:(s + 1) * 128, :], in_=o_t)

            # ---- phase A: chunk groups with strip admission ----------------
            # DMA need-order on the sync queue: g one chunk ahead, w right
            # behind its g, x strips in the slack. Chunk 0's mask pipe is
            # split into o-halves to cut the initial PE bubble.
            # admission chunk per strip; the phase-A DMA cycle must not exceed
            # the ~7.4us DVE mask pace, so only cycles 0-3 carry an x strip
            # (cycles 4-7 let the g stream catch up); strips 5-7 admit at
            # chunk 7 via full catch-up bursts fed by post-g7 x loads
            admit_at = [0, 1, 1, 2, 3, 4, 5, 6]
            x_cycle = {0: [1], 1: [2], 2: [3], 3: [4], 4: [5], 5: [6], 6: [7]}
            Q = O // 4
            if mode == "const":
                for q in range(4):
                    g_dma(0, slice(q * Q, (q + 1) * Q))
                    if q == 0:
                        w_dma(0)
                x_dma(0)
                g_dma(1)
                w_dma(1)
                for q in range(4):
                    mask_unit(0, slice(q * Q, (q + 1) * Q))
            else:
                g_dma(0)
                w_dma(0)
                x_dma(0)
                g_dma(1)
                w_dma(1)
                mask_unit(0)
            for bt in range(N_BT):
                if bt > 0:
                    mask_unit(bt)
                for s in x_cycle.get(bt, []):
                    x_dma(s)
                if bt + 2 < N_BT:
                    g_dma(bt + 2)
                    w_dma(bt + 2)
                elif bt + 1 == N_BT:
                    for s in range(N_BT, N_ADMIT + 2):
                        x_dma(s)
                for s in range(N_ADMIT):
                    if admit_at[s] < bt:
                        burst(s, bt)
                for s in range(N_ADMIT):
                    if admit_at[s] == bt:
                        for c in range(bt + 1):
                            burst(s, c)

            # ---- phase B: drain residents, stream remaining strips ---------
            for s in range(N_ADMIT):
                drain(s)
            for s in range(N_ADMIT, N_TT - 1):
                if N_ADMIT + 2 <= s + 2 <= N_TT - 1:
                    x_dma(s + 2)
                for bt in range(N_BT):
                    burst(s, bt)
                drain(s)

            # last strip: run o-quarters as sequential psum groups so each
            # quarter's drain+store hides under the next quarter's matmuls
            s = N_TT - 1
            x_dma(s)
            xs = x_tiles[s]
            acc = ps_pool.tile([128, O], f32, tag="acc", name="acclast")
            o_t = out_pool.tile([128, O], bf16, tag="o", name="olast")
            Hd = O // 2
            for ch, csl in enumerate(slice(q * Hd, (q + 1) * Hd)
                                     for q in range(2)):
                n = 0
                ntot = insts_per_strip
                for bt in range(N_BT):
                    pairs = [(bt * 4 + 2 * h, wm8[bt]) for h in (0, 1)]
                    if bt < cov_x:
                        pairs += [(32 + bt * 4 + 2 * h, wm8[bt]) for h in (0, 1)]
                    if bt < cov_w:
                        pairs += [(bt * 4 + 2 * h, wmr[bt]) for h in (0, 1)]
                    for xoff, rhs_t in pairs:
                        h2 = (xoff % 4) // 2 * 2
                        nc.tensor.matmul(
                            acc[:, csl],
                            xs[:, xoff:xoff + 2, :],
                            rhs_t[:, h2:h2 + 2, csl],
                            start=(n == 0),
                            stop=(n == ntot - 1),
                            perf_mode=DR,
                        )
                        n += 1
                nc.vector.tensor_add(o_t[:, csl], acc[:, csl], bias_s[:, csl])
                nc.scalar.dma_start(
                    out=out_d[s * 128:(s + 1) * 128, csl], in_=o_t[:, csl])

    nc.compile()
    return nc


def _get_program(mode, const_c=None):
    key = (mode, COV_X, COV_W)
    if key not in _prog_cache:
        _prog_cache[key] = _build_program(mode, COV_X, COV_W)
    return _prog_cache[key]


def pack_inputs(x, weight, bias, choice_weights, gumbel_noise):
    """Host-side prep: returns (mode, per-core input maps)."""
    import ml_dtypes

    E4 = ml_dtypes.float8_e4m3
    BF = ml_dtypes.bfloat16

    x = np.asarray(x, dtype=np.float32).reshape(T, K)
    w = np.asarray(weight, dtype=np.float32)
    b = np.asarray(bias, dtype=np.float32).reshape(1, O_FULL)
    g = np.asarray(gumbel_noise, dtype=np.float32)
    cw = np.asarray(choice_weights, dtype=np.float32)

    is_const = bool((cw == cw.flat[0]).all())
    mode = "const" if is_const else "full"

    # x planes, k reordered to (bt, bi, p): k = 4*(bt*128+bi) + p
    xs = x * SX
    x8 = xs.astype(E4)
    xr = (xs - x8.astype(np.float32)).astype(E4)
    NKX = 32 + 4 * COV_X

    def pack_x(plane, nbt):
        # [T, K] -> [s, t, bt, bi, p] -> [bi, s, bt, p, t]
        a = plane.reshape(N_TT, 128, N_BT, 128, 4)[:, :, :nbt]
        return a.transpose(3, 0, 2, 4, 1)  # [bi, s, bt, p, t]

    x8p = pack_x(x8, N_BT).reshape(128, N_TT, 32, 128)
    xt = np.empty((128, N_TT, NKX, 128), dtype=E4)
    xt[:, :, :32] = x8p
    if COV_X:
        xt[:, :, 32:] = pack_x(xr, COV_X).reshape(128, N_TT, 4 * COV_X, 128)
    xt = np.ascontiguousarray(xt).reshape(128, N_TT * NKX * 128)

    # weight planes (unmasked; device masks them), k'' order, [k'', p->, o]
    ws = w * SW
    w8 = ws.astype(E4)
    wr = (ws - w8.astype(np.float32)).astype(E4)

    def pack_w(plane, rows, nbt):
        # [O, K] slice -> [o, bt, bi, p] -> [bt, bi, p, o]
        a = plane[rows].reshape(O, N_BT, 128, 4)[:, :nbt]
        a = a.transpose(1, 2, 3, 0)  # [bt, bi, p, o]
        return np.ascontiguousarray(a).reshape(nbt * 128, 4 * O)

    # gumbel planar per chunk with plane order [g4,g3,g2,g1,g0,g5]
    PERM = [4, 3, 2, 1, 0, 5]

    def pack_g(a, rows):
        # [O_FULL*K/4, 6] -> rows -> [o, bt, bi, jperm] -> [bt, bi, j, o]
        a = a.reshape(O_FULL, K // 4, 6)[rows][:, :, PERM]
        a = a.reshape(O, N_BT, 128, 6).transpose(1, 2, 3, 0)
        return np.ascontiguousarray(a).reshape(N_BT * 128, 6 * O)

    in_maps = []
    for c in range(N_CORES):
        rows = slice(c * O, (c + 1) * O)
        m = {
            "xt": xt,
            "w8": pack_w(w8, rows, N_BT),
            "wr": pack_w(wr, rows, COV_W),
            "g": pack_g(g, rows),
            "b": np.ascontiguousarray(b[:, rows] * S),
        }
        if mode == "full":
            m["cw"] = pack_g(cw, rows)
        in_maps.append(m)
    return mode, in_maps


def kernel(x, weight, bias, choice_weights, gumbel_noise):
    from concourse.bass_utils import run_bass_kernel_spmd

    mode, in_maps = pack_inputs(x, weight, bias, choice_weights, gumbel_noise)
    nc = _get_program(mode)
    res = run_bass_kernel_spmd(nc, in_maps, list(range(N_CORES)))
    parts = [res.results[c]["out"].astype(np.float32) * (1.0 / S)
             for c in range(N_CORES)]
    out = np.concatenate(parts, axis=1)  # [T, O_FULL]
    return out.reshape(2, 2048, O_FULL)
